# revision 60
# baseline (speedup 1.0000x reference)
"""Trainium2 Bass kernel for nn_Compressor (sparse_attention block compressor).

Math (reference):
  proj = x @ [W_kv; W_gate]^T            # [b*s, 2048]
  kv   = proj[:, :1024] + ape[s%4]       # blockwise (RATIO=4) abs-pos bias
  sc   = proj[:, 1024:]
  window(blk) = {prev blk rows, ch 0:512} + {cur blk rows, ch 512:1024}
  pooled[blk, c] = softmax-gated channelwise pool over the 8-entry window
  out = (RMSNorm(pooled) -> rope on ch 448:512) @ H  (512x512 Hadamard)

Distribution: 8 cores, data-parallel over (batch, seq-half). Each core owns
2048 seq rows = 512 blocks; the 1-block halo is handled by shifting the
matmul rhs window by 4 rows (xs input carries 16 halo rows).

Implementation notes:
  * Tensor-bound problem: 2048 proj matmuls/core at ~216ns is ~94% of the
    wall time; everything else is ramp/tail/overlap engineering.
  * x^T bf16 prepared and pre-tiled on host (truncate hi-16 of f32);
    single-d-chunk DMA descriptors (135KB, ~3.5us latency each) on the
    dedicated gpsimd queue give fine-grained deps so matmuls chase the DMA.
  * Weight tiles stream on the sync queue as quarter descriptors (a 1MB
    single descriptor has ~20us latency - per-descriptor DMA throughput is
    only ~40GB/s; parallelism comes from many descriptors in flight).
  * Big consts ride the gpsimd queue BEHIND chunk-0's x tiles (per-queue
    FIFO keeps ramp-critical bytes first); dma_start costs ~700ns of
    descriptor-gen on the issuing engine, so hot queues stay lean.
  * Projections: W^T stationary (lhsT), x^T moving -> PSUM [och, rows]; halo
    handled by a free-axis offset (12 vs 16) of the rhs window.
  * Softmax without max-subtraction; block-0 masking via 0/1 multiply.
  * All epilogue matmuls in bf16 (fp32 matmul is 4x slower): Hadamard and
    RMSNorm-variance accumulate per-group-j inside the stream; variance
    comes out as a PSUM column via lhsT=sq matmul (no transpose needed).
  * Per-block scale = 1/sqrt(var) applied on the Hadamard output rows
    (scale commutes with the per-block-linear Hadamard/rope), split across
    Scalar+Vector halves to shorten the tail.
  * reciprocal_approx_fast (~18 bits) replaces the slow DVE reciprocal; a
    dummy Sqrt after the last Exp preloads the scalar activation table off
    the critical path.
  * Evictions: exp on Scalar, kv+ape on Vector; pooling on Vector; rope
    group runs first on the last chunk so the closing chain is short.
  * fp8 DoubleRow (2x matmul rate) was evaluated and rejected: e4m3
    quantization of x or W costs 3-5% output error vs the 2% budget, and
    hi/lo-split fp8 costs the same as bf16.
"""

import os
import numpy as np
import ml_dtypes

import concourse.bass as bass
import concourse.bacc as bacc
import concourse.mybir as mybir
from concourse.tile import TileContext
from concourse.bass_utils import run_bass_kernel_spmd

BF16 = ml_dtypes.bfloat16
F32 = mybir.dt.float32
BF = mybir.dt.bfloat16

N_CORES = 8
DIM = 4096
OCH = 2048          # kv 1024 + gate 1024
ROWS = 2048         # own rows per core
XS_ROWS = 2064      # 16 halo/pad rows + 2048
MCH = 4             # m-chunks per core
MROWS = 512         # rows per m-chunk
NBLK = 128          # blocks per m-chunk
DCH = 32            # d chunks of 128
OCHK = 16           # o chunks of 128
# o-chunks 0..3 kv-first(prev), 4..7 kv-second(cur), 8..11 sc-first, 12..15 sc-second
FIRST_HALF = (0, 1, 2, 3, 8, 9, 10, 11)

_CACHE = {}


def _build():
    nc = bacc.Bacc("TRN2", target_bir_lowering=False, debug=False,
                   num_devices=N_CORES)
    # host-pre-tiled x^T: [mch, 4d-group, part, d-in-group, m(528)]
    xs = nc.dram_tensor("xs", [MCH, DCH // 4, 128, 4, 528], BF,
                        kind="ExternalInput")
    wp = nc.dram_tensor("wp", [OCHK, 128, DCH, 128], BF, kind="ExternalInput")
    ape_d = nc.dram_tensor("ape_t", [128, 32], F32, kind="ExternalInput")
    cos_d = nc.dram_tensor("cos_t", [128, 512], F32, kind="ExternalInput")
    sin_d = nc.dram_tensor("sin_t", [128, 512], F32, kind="ExternalInput")
    psw_d = nc.dram_tensor("psw", [128, 128], BF, kind="ExternalInput")
    h_d = nc.dram_tensor("hmat", [128, 4, 512], BF, kind="ExternalInput")
    zmask_d = nc.dram_tensor("zmask", [128, 1], F32, kind="ExternalInput")
    out_d = nc.dram_tensor("out", [4 * NBLK, 512], F32, kind="ExternalOutput")

    X = mybir.AxisListType.X

    with TileContext(nc) as tc:
        with (
            tc.tile_pool(name="const", bufs=1) as constp,
            tc.tile_pool(name="xt", bufs=2) as xtp,
            tc.tile_pool(name="wt", bufs=3) as wtp,
            tc.tile_pool(name="sb", bufs=2) as sbp,
            tc.tile_pool(name="pl", bufs=2) as plp,
            tc.tile_pool(name="sm", bufs=2) as smp,
            tc.tile_pool(name="osb", bufs=2) as outp,
            tc.tile_pool(name="proj", bufs=5, space="PSUM") as projp,
            tc.tile_pool(name="had", bufs=1, space="PSUM") as hadp,
            tc.tile_pool(name="vc", bufs=1, space="PSUM") as vcolp,
            tc.tile_pool(name="sw", bufs=1, space="PSUM") as swp,
        ):
            # Queues: w=sync, xt=gpsimd+scalar (split halves the ~640ns
            # per-dma_start descriptor-gen cost), consts+out=scalar (after
            # the first xt batch). Vector cannot issue DMAs.
            zmask_sb = constp.tile([128, 1], F32, tag="zmask")
            ape_sb = constp.tile([128, 32], F32, tag="ape")
            h_sb = constp.tile([128, 4, 512], BF, tag="h")
            psw_sb = constp.tile([128, 128], BF, tag="psw")
            cos_sb = constp.tile([128, 512], F32, tag="cos")
            sin_sb = constp.tile([128, 512], F32, tag="sin")
            ones_sb = constp.tile([128, 1], BF, tag="ones")
            nc.vector.memset(ones_sb[:], 1.0)
            eps_sb = constp.tile([128, 1], F32, tag="eps")
            nc.vector.memset(eps_sb[:], 1e-6)

            def load_small_consts():
                nc.scalar.dma_start(out=zmask_sb[:], in_=zmask_d[:, :])
                nc.scalar.dma_start(out=ape_sb[:], in_=ape_d[:, :])

            def load_big_consts():
                # behind chunk-0's xt on the gpsimd queue: per-queue FIFO
                # data order keeps the ramp-critical x bytes ahead
                nc.gpsimd.dma_start(out=h_sb[:], in_=h_d[:, :, :])
                nc.gpsimd.dma_start(out=psw_sb[:], in_=psw_d[:, :])
                nc.gpsimd.dma_start(out=cos_sb[:], in_=cos_d[:, :])
                nc.gpsimd.dma_start(out=sin_sb[:], in_=sin_d[:, :])

            def g4(tile_ap):
                return tile_ap.rearrange("p (b r) -> p b r", r=4)

            for mch in range(MCH):
                r0 = MROWS * mch
                # x^T tiles of 4 d-chunks each: [128, 4, 528] bf16 (slot s <->
                # own row r0 + s - 16; slots 12..15 = halo). 8 descriptors
                # per chunk keeps the ~700ns/dma_start issue cost small while
                # the 4-d granularity still lets matmuls chase the DMA.
                # single-d descriptors (135KB, ~3.5us single-descriptor
                # latency) on the dedicated gpsimd queue
                if mch == 0:
                    load_small_consts()
                    # very first weight tile as 4 independent tiles: per-part
                    # deps let the first matmuls start on quarter 0 (~9.6us)
                    # instead of the whole 1MB tile (~12us)
                    w8p = []
                    for q in range(4):
                        wq = constp.tile([128, 8, 128], BF, tag=f"w8p{q}")
                        nc.sync.dma_start(
                            out=wq[:], in_=wp[8, :, 8 * q:8 * (q + 1), :])
                        w8p.append(wq)
                xq = []
                for c in range(DCH // 4):
                    t = xtp.tile([128, 4, 528], BF, tag=f"xt{c}")
                    for i in range(4):
                        if mch == 0 and c == 0 and i == 0:
                            # halve the very first descriptor: mm0 waits on
                            # it, and per-descriptor DMA is only ~40GB/s
                            nc.gpsimd.dma_start(out=t[:, 0, 0:264],
                                                in_=xs[0, 0, :, 0, 0:264])
                            nc.gpsimd.dma_start(out=t[:, 0, 264:528],
                                                in_=xs[0, 0, :, 0, 264:528])
                        else:
                            nc.gpsimd.dma_start(out=t[:, i, :],
                                                in_=xs[mch, c, :, i])
                    xq.append(t)
                if mch == 0:
                    load_big_consts()

                pooled = plp.tile([128, 4, NBLK], BF, tag="pooled")
                had_ps = hadp.tile([128, 512], F32, tag="had")
                vcol_ps = vcolp.tile([128, 1], F32, tag="vcol")

                # rope (group 3) first on the last chunk, so the closing
                # chain after the final proj matmul is as short as possible
                jorder = (3, 0, 1, 2) if mch == MCH - 1 else (0, 1, 2, 3)
                for jn, j in enumerate(jorder):
                    group = {}
                    # scores first so exp/reduce overlap the kv matmuls
                    # the kernel's very last proj tile: do it in column
                    # halves so eviction/pooling of half A overlaps half B's
                    # matmuls, halving the closing dependency chain
                    split_last = mch == MCH - 1 and jn == 3
                    for t, oc in enumerate((j + 8, j + 12, j, j + 4)):
                        first_tile = mch == 0 and jn == 0 and t == 0
                        if not first_tile:
                            # quarter descriptors: ~6.5us single-descriptor
                            # latency each vs ~20us for the whole 1MB tile
                            w = wtp.tile([128, DCH, 128], BF, tag="w")
                            for q in range(4):
                                nc.sync.dma_start(
                                    out=w[:, 8 * q:8 * (q + 1), :],
                                    in_=wp[oc, :, 8 * q:8 * (q + 1), :])
                        ps = projp.tile([128, MROWS], F32, tag="proj")
                        off = 12 if oc in FIRST_HALF else 16
                        halves = ((0, 256), (256, 512)) if (
                            split_last and t == 3) else ((0, MROWS),)
                        for o0, o1 in halves:
                            for d in range(DCH):
                                lhsT = (w8p[d // 8][:, d % 8, :] if first_tile
                                        else w[:, d, :])
                                nc.tensor.matmul(
                                    ps[:, o0:o1],
                                    lhsT=lhsT,
                                    rhs=xq[d // 4][:, d % 4,
                                                   off + o0:off + o1],
                                    start=(d == 0),
                                    stop=(d == DCH - 1),
                                )
                        if oc >= 8:
                            # score chunk: e = exp(psum) -> bf16 SBUF
                            k = 1 if oc < 12 else 2
                            e = sbp.tile([128, MROWS], BF, tag=f"e{k}")
                            nc.scalar.activation(
                                e[:], ps[:], mybir.ActivationFunctionType.Exp)
                            if mch == 0 and oc < 12:
                                # block-0 of even cores: zero the 4 prev-window
                                # weights (zmask = 0 even / 1 odd)
                                nc.vector.tensor_scalar_mul(
                                    e[:, 0:4], e[:, 0:4], zmask_sb[:, 0:1])
                            group[f"e{k}"] = e
                        else:
                            # kv chunk: PSUM -> bf16 SBUF with ape bias added
                            k = 1 if oc < 4 else 2
                            kv = sbp.tile([128, MROWS], BF, tag=f"kv{k}")
                            for o0, o1 in halves:
                                nb = (o1 - o0) // 4
                                ape_ap = (ape_sb[:, 4 * oc:4 * oc + 4]
                                          .unsqueeze(1)
                                          .to_broadcast((128, nb, 4)))
                                nc.vector.tensor_add(
                                    g4(kv[:, o0:o1]), g4(ps[:, o0:o1]), ape_ap)
                            group[f"kv{k}"] = kv

                    kv1, kv2 = group["kv1"], group["kv2"]
                    e1, e2 = group["e1"], group["e2"]

                    # window reduces via avg-pool (innermost dim of 4); the
                    # /4 cancels between qsum and ssum
                    s1 = smp.tile([128, NBLK], F32, tag="s1")
                    nc.vector.reduce_sum(s1[:], g4(e1[:]), axis=X)
                    s2 = smp.tile([128, NBLK], F32, tag="s2")
                    nc.vector.reduce_sum(s2[:], g4(e2[:]), axis=X)
                    ssum = smp.tile([128, NBLK], F32, tag="ssum")
                    nc.vector.tensor_add(ssum[:], s1[:], s2[:])
                    rinv = smp.tile([128, NBLK], F32, tag="rinv")
                    nc.vector.reciprocal_approx_fast(rinv[:], ssum[:])

                    pm = sbp.tile([128, MROWS], BF, tag="pm")
                    nc.vector.tensor_mul(pm[:], e1[:], kv1[:])
                    q1 = smp.tile([128, NBLK], F32, tag="q1")
                    nc.vector.reduce_sum(q1[:], g4(pm[:]), axis=X)
                    pm2 = sbp.tile([128, MROWS], BF, tag="pm2")
                    q2 = smp.tile([128, NBLK], F32, tag="q2")
                    kv2_halves = ((0, 256), (256, 512)) if split_last \
                        else ((0, MROWS),)
                    for o0, o1 in kv2_halves:
                        nc.vector.tensor_mul(pm2[:, o0:o1], e2[:, o0:o1],
                                             kv2[:, o0:o1])
                        nc.vector.reduce_sum(q2[:, o0 // 4:o1 // 4],
                                             g4(pm2[:, o0:o1]), axis=X)
                    qsum = smp.tile([128, NBLK], F32, tag="qsum")
                    nc.vector.tensor_add(qsum[:], q1[:], q2[:])

                    nc.vector.tensor_mul(pooled[:, j, :], qsum[:], rinv[:])

                    # RMSNorm variance: vcol[b] += sum_c pooled[c,b]^2
                    # (square on DVE; the ns matmul overlaps the stream)
                    sq = smp.tile([128, NBLK], BF, tag="sq")
                    nc.vector.tensor_mul(sq[:], pooled[:, j, :], pooled[:, j, :])
                    nc.tensor.matmul(vcol_ps[:], lhsT=sq[:], rhs=ones_sb[:, 0:1],
                                     start=(jn == 0), stop=(jn == 3))

                    if j == 3:
                        # rope on chunk 3 (ch 384..511; rows 64.. are rope)
                        sw_ps = swp.tile([128, NBLK], F32, tag="swap")
                        nc.tensor.matmul(sw_ps[:], lhsT=psw_sb[:],
                                         rhs=pooled[:, 3, :],
                                         start=True, stop=True)
                        cslice = cos_sb[:, mch * NBLK:(mch + 1) * NBLK]
                        sslice = sin_sb[:, mch * NBLK:(mch + 1) * NBLK]
                        tmpc = smp.tile([128, NBLK], F32, tag="tmpc")
                        nc.vector.tensor_mul(tmpc[:], pooled[:, 3, :], cslice)
                        tmps = smp.tile([128, NBLK], F32, tag="tmps")
                        nc.vector.tensor_mul(tmps[:], sw_ps[:], sslice)
                        nc.vector.tensor_add(pooled[:, 3, :], tmpc[:], tmps[:])
                    nc.tensor.matmul(had_ps[:], lhsT=pooled[:, j, :],
                                     rhs=h_sb[:, j, :],
                                     start=(jn == 0), stop=(jn == 3))

                    if jn == 3:
                        # preload the Sqrt activation table while pooling of
                        # the final group is still in flight (depend on e2 so
                        # the scheduler keeps this after the last Exp)
                        dummy = smp.tile([1, 1], F32, tag="dummy")
                        nc.scalar.activation(
                            dummy[:], e2[0:1, 0:1],
                            mybir.ActivationFunctionType.Sqrt)

                # ---- scale column + output ----
                sd = smp.tile([128, 1], F32, tag="sd")
                nc.scalar.activation(sd[:], vcol_ps[:],
                                     mybir.ActivationFunctionType.Sqrt,
                                     scale=1.0 / 512.0, bias=eps_sb[:, 0:1])
                scol = smp.tile([128, 1], F32, tag="scol")
                nc.vector.reciprocal_approx_fast(scol[:], sd[:])
                out_sb = outp.tile([128, 512], F32, tag="out")
                # scale-apply split across engines to halve the tail latency
                nc.scalar.activation(out_sb[:, 0:256], had_ps[:, 0:256],
                                     mybir.ActivationFunctionType.Copy,
                                     scale=scol[:, 0:1])
                nc.vector.tensor_scalar_mul(
                    out_sb[:, 256:512], had_ps[:, 256:512], scol[:, 0:1])
                nc.scalar.dma_start(
                    out=out_d[mch * NBLK:(mch + 1) * NBLK, :], in_=out_sb[:])
    nc.compile()
    return nc


def _prep_shared(W_kv, W_gate, ape, norm_w, H):
    W = np.concatenate([W_kv, W_gate], axis=0).astype(np.float32)  # [2048, 4096]
    Wb = W.astype(BF16)
    wp = np.ascontiguousarray(
        Wb.T.reshape(DCH, 128, OCHK, 128).transpose(2, 1, 0, 3))  # [16,128,32,128]
    ape_t = np.ascontiguousarray(
        ape.astype(np.float32).T.reshape(8, 128, 4).transpose(1, 0, 2)
    ).reshape(128, 32)
    psw = np.zeros((128, 128), np.float32)
    idx = np.arange(64)
    psw[idx, idx] = 1.0
    k2 = np.arange(0, 64, 2)
    psw[64 + k2 + 1, 64 + k2] = 1.0
    psw[64 + k2, 64 + k2 + 1] = 1.0
    hm = np.ascontiguousarray(
        (norm_w.astype(np.float32)[:, None] * H.astype(np.float32))
        .reshape(4, 128, 512).transpose(1, 0, 2))
    return wp, ape_t, psw.astype(BF16), hm.astype(BF16)


def _hadamard(n):
    h = np.array([[1.0]], dtype=np.float32)
    while h.shape[0] < n:
        h = np.block([[h, h], [h, -h]])
    return (h / np.sqrt(n)).astype(np.float32)


def _make_in_maps(x, W_kv, W_gate, ape, norm_w, freqs_cis):
    b, s, _ = x.shape
    H = _hadamard(512)
    wp, ape_t, psw, hm = _prep_shared(W_kv, W_gate, ape, norm_w, H)

    # truncate-to-bf16 (hi-16 planes of the f32 words) and transpose once
    xh = x.reshape(b * s, DIM).view(BF16)[:, 1::2]
    xT = np.ascontiguousarray(xh.T)  # [4096, 16384]
    fr = freqs_cis[:, :, 0]  # [nb, 32]
    fi = freqs_cis[:, :, 1]

    in_maps = []
    for c in range(N_CORES):
        batch, half = c // 2, c % 2
        R0 = batch * s + half * ROWS
        xsf = np.zeros((DIM, XS_ROWS), BF16)
        xsf[:, 16:] = xT[:, R0:R0 + ROWS]
        if half == 1:
            xsf[:, :16] = xT[:, R0 - 16:R0]
        # pre-tile: [mch, 4d-group, part, d-in-group, 528]
        v = xsf.reshape(DCH, 128, XS_ROWS)
        xs = np.stack([
            v[:, :, 512 * m:512 * m + 528]
            .reshape(8, 4, 128, 528).transpose(0, 2, 1, 3)
            for m in range(MCH)])
        xs = np.ascontiguousarray(xs)

        g0 = half * 512
        bi = np.arange(g0, g0 + 512)
        cos_t = np.zeros((128, 512), np.float32)
        cos_t[:64] = 1.0
        cos_t[64:] = np.repeat(fr[bi].T, 2, axis=0)
        sin_t = np.zeros((128, 512), np.float32)
        st = np.repeat(fi[bi].T, 2, axis=0)
        st[0::2] *= -1.0
        sin_t[64:] = st

        zmask = np.full((128, 1), 0.0 if half == 0 else 1.0, np.float32)
        in_maps.append({
            "xs": xs, "wp": wp, "ape_t": ape_t,
            "cos_t": cos_t, "sin_t": sin_t, "psw": psw,
            "hmat": hm, "zmask": zmask,
        })
    return in_maps


def kernel(x, W_kv, W_gate, ape, norm_w, freqs_cis, start_pos=0):
    x = np.asarray(x, dtype=np.float32)
    W_kv = np.asarray(W_kv, dtype=np.float32)
    W_gate = np.asarray(W_gate, dtype=np.float32)
    ape = np.asarray(ape, dtype=np.float32)
    norm_w = np.asarray(norm_w, dtype=np.float32)
    freqs_cis = np.asarray(freqs_cis, dtype=np.float32)

    b, s, _ = x.shape
    nb = s // 4
    assert (b, s) == (4, 4096), (b, s)

    if "nc" not in _CACHE:
        _CACHE["nc"] = _build()
    nc = _CACHE["nc"]

    in_maps = _make_in_maps(x, W_kv, W_gate, ape, norm_w, freqs_cis)

    trace = os.environ.get("KERNEL_TRACE", "") not in ("", "0")
    res = run_bass_kernel_spmd(nc, in_maps, core_ids=list(range(N_CORES)),
                               trace=trace)
    kernel.last_results = res
    out = np.concatenate([res.results[c]["out"] for c in range(N_CORES)], axis=0)
    return np.ascontiguousarray(out.reshape(b, nb, 512))
